# revision 1
# baseline (speedup 1.0000x reference)
"""Self-contained Trainium2 Bass kernel for nn_DiffusionLoss_56719338111476.

kernel(**inputs) takes the FULL unsharded inputs and returns the full scalar
output. Internally: 8-way shard (batch x row-block of the pairwise matrix),
one SPMD Bass/Tile kernel run on cores 0-7 via bass_utils.run_bass_kernel_spmd,
tiny final assembly (3x3 SVD rigid align, bond denominator, diagonal
corrections) on host in float64.
"""
import numpy as np
from contextlib import ExitStack


B, NA, NT = 2, 2048, 256
T = 4.0
SIGMA_DATA = 16.0
ALPHA_BOND = 1.0
ALPHA_DNA, ALPHA_RNA, ALPHA_LIGAND = 5.0, 5.0, 10.0
WT = (T**2 + SIGMA_DATA**2) / (T + SIGMA_DATA) ** 2

N_CORES = 8
ROWS_PER_CORE = NA // 4  # 512
SUBS = 4                 # 128-row subblocks per core
EPS = 4e-3               # folded into squared distances (keeps sqrt args > 0)
BIG = 32768.0            # c-gate additive (exact in bf16)
BIGD2 = 1.0e8            # added to dxgt^2 for masked-out columns -> dxgt ~ 1e4
THRS = (0.5, 1.0, 2.0, 4.0)
SIG3A = (0.24310539, 0.50959069, 0.24941822)
SIG3B = (1.00862224, 0.9749538, 1.00085966)
SIG3C = (2.00029746, 0.75504458, 4.0009847)
NSIG = 3

# out layout per core: [128, 16] f32
#  cols 0:12   sigma sums   (sub*3 + i), amplitudes applied on host
#  cols 12:16  c-count per row (ge-count)
# outb [1, 512] f32: bond partial column sums
OUT_COLS = 16


def sigmoid(x):
    return 1.0 / (1.0 + np.exp(-np.clip(x, -60, 60)))


def pack_inputs(x, x_gt, atom_mask, A, token_bonds, is_polymer, is_ligand,
                is_dna, is_rna):
    """Returns (in_maps, host_ctx). in_maps: list of 8 dicts of np arrays."""
    import ml_dtypes
    bf16 = ml_dtypes.bfloat16

    x = np.asarray(x, np.float32)
    x_gt = np.asarray(x_gt, np.float32)
    atom_mask = np.asarray(atom_mask, np.float32)
    A = np.asarray(A, np.float32)
    token_bonds = np.asarray(token_bonds, np.float32)
    is_polymer = np.asarray(is_polymer, np.float32)
    is_ligand = np.asarray(is_ligand, np.float32)
    is_dna = np.asarray(is_dna, np.float32)
    is_rna = np.asarray(is_rna, np.float32)

    in_maps = []
    ctx = {"atom_mask": atom_mask}

    # per-batch tensors
    btok_m = token_bonds * (is_polymer[:, None, :] * is_ligand[:, :, None])  # [B,NT,NT]
    Am = A * atom_mask[:, :, None]              # [B,NA,NT]
    AmT = np.swapaxes(Am, 1, 2)                 # [B,NT,NA]
    is_nuc = np.einsum('bat,bt->ba', A, is_dna + is_rna)  # [B,NA]
    thr = np.where(is_nuc > 0.5, 30.0, 15.0).astype(np.float32)  # [B,NA]

    ctx["btok_m"] = btok_m
    ctx["Am"] = Am
    ctx["thr"] = thr

    # bf16 hi/lo split of coordinates; represented x~ = xh + xl
    xh = x.astype(bf16).astype(np.float32)
    xl = (x - xh).astype(bf16).astype(np.float32)
    gh = x_gt.astype(bf16).astype(np.float32)
    gl = (x_gt - gh).astype(bf16).astype(np.float32)
    xt = xh.astype(np.float64) + xl.astype(np.float64)   # [B,NA,3]
    gtt = gh.astype(np.float64) + gl.astype(np.float64)
    nx = np.sum(xt * xt, -1)       # [B,NA] f64
    ng = np.sum(gtt * gtt, -1)

    def split3(v):
        v = v.copy()
        parts = []
        for _ in range(3):
            p = v.astype(np.float32).astype(bf16).astype(np.float64)
            parts.append(p.astype(np.float32))
            v = v - p
        return parts

    def mk_lhs(h, l, b, rows):
        out = np.ones((15, ROWS_PER_CORE), np.float32)
        out[0:3] = h[b, rows].T
        out[3:6] = l[b, rows].T
        out[6:9] = h[b, rows].T
        out[9:12] = l[b, rows].T
        return out.astype(bf16)

    def mk_rhs(h, l, nbv, b):
        out = np.zeros((15, NA), np.float32)
        out[0:3] = -2.0 * h[b].T
        out[3:6] = -2.0 * h[b].T
        out[6:9] = -2.0 * l[b].T
        out[9:12] = -2.0 * l[b].T
        p = split3(nbv)
        out[12], out[13], out[14] = p[0], p[1], p[2]
        return out.astype(bf16)

    for c in range(N_CORES):
        b = c // 4
        r0 = (c % 4) * ROWS_PER_CORE
        rows = slice(r0, r0 + ROWS_PER_CORE)

        nax = (nx[b, rows].astype(np.float32) + EPS).reshape(SUBS, 128).T
        nag = (ng[b, rows].astype(np.float32) + EPS).reshape(SUBS, 128).T
        thrpk = thr[b, rows].reshape(SUBS, 128).T.copy()

        in_maps.append(dict(
            lhsx=mk_lhs(xh, xl, b, rows),
            lhsg=mk_lhs(gh, gl, b, rows),
            rhsx=mk_rhs(xh, xl, nx[b], b),
            rhsg=mk_rhs(gh, gl, ng[b] + BIGD2 * (1.0 - atom_mask[b]), b),
            nax=np.ascontiguousarray(nax.astype(np.float32)),
            nag=np.ascontiguousarray(nag.astype(np.float32)),
            amt=AmT[b].astype(bf16),
            amt_own=AmT[b, :, rows].astype(bf16),
            btok=btok_m[b].astype(bf16),
            thrpk=np.ascontiguousarray(thrpk),
        ))
    return in_maps, ctx


def emulate_device(in_map):
    """Numpy mirror of the device program for one core.

    Returns dict(out=[128, OUT_COLS], outb=[4, 512])."""
    out = np.zeros((128, OUT_COLS), np.float32)
    outb = np.zeros((1, 512), np.float32)
    lhsx = np.asarray(in_map["lhsx"], np.float32)
    lhsg = np.asarray(in_map["lhsg"], np.float32)
    rhsx = np.asarray(in_map["rhsx"], np.float32)
    rhsg = np.asarray(in_map["rhsg"], np.float32)
    nax, nag = in_map["nax"], in_map["nag"]
    amt = np.asarray(in_map["amt"], np.float32)
    amt_own = np.asarray(in_map["amt_own"], np.float32)
    btok = np.asarray(in_map["btok"], np.float32)
    thrpk = in_map["thrpk"]
    import ml_dtypes
    bf = ml_dtypes.bfloat16

    for s in range(SUBS):
        cols = slice(s * 128, (s + 1) * 128)
        pa = lhsx[:, cols].T @ rhsx + nax[:, s:s + 1]   # [128, NA] dx^2 (+eps)
        pb = lhsg[:, cols].T @ rhsg + nag[:, s:s + 1]
        dx = np.sqrt(pa)
        dg = np.sqrt(pb)
        df = (dx - dg).astype(bf).astype(np.float32)
        qc = (dg >= thrpk[:, s:s + 1]).astype(np.float32)
        out[:, 12 + s] = qc.sum(-1)
        dabs = np.abs(df)
        dp = (dabs + qc * BIG).astype(bf).astype(np.float32)
        for k in range(NSIG):
            sg = sigmoid(SIG3B[k] * (SIG3C[k] - dp))
            out[:, s * NSIG + k] = sg.sum(-1)
        # bond
        rb = (amt_own[:, cols].T @ btok).T   # [NT, 128] row_bondT
        dabs_bf = dabs.astype(bf).astype(np.float32)
        for j in range(4):
            jc = slice(j * 512, (j + 1) * 512)
            pc = rb.T @ amt[:, jc]
            h = (dabs_bf[:, jc] * pc).astype(bf).astype(np.float32)
            h2 = (h * h).astype(bf).astype(np.float32)
            outb[0] += h2.sum(0)
    return dict(out=out, outb=outb)


def _weighted_rigid_align_np(xp, xp_gt, w, mask):
    # all [NA,...] for one batch element; float64
    n = mask.sum()
    w_mean = (w * mask).sum() / n
    wm = (w * mask)[:, None]
    mu = (xp * wm).sum(0) / n / w_mean
    mu_gt = (xp_gt * wm).sum(0) / n / w_mean
    xc = xp - mu
    xgc = xp_gt - mu_gt
    H = np.einsum('ni,nj,n->ij', xgc, xc, w * mask)
    U, _, Vh = np.linalg.svd(H)
    d = np.sign(np.linalg.det(U @ Vh))
    F = np.diag([1.0, 1.0, d])
    R = U @ F @ Vh
    return xc @ R.T + mu_gt


def assemble(outs, inputs, ctx):
    """outs: list of 8 [128, OUT_COLS] device results. Returns final scalar."""
    x = np.asarray(inputs["x"], np.float64)
    x_gt = np.asarray(inputs["x_gt"], np.float64)
    atom_mask = np.asarray(ctx["atom_mask"], np.float64)
    A = np.asarray(inputs["atom_to_token_index"], np.float64)
    btok_m = np.asarray(ctx["btok_m"], np.float64)
    Am = np.asarray(ctx["Am"], np.float64)

    # diag e (d'=0), in the fitted basis
    sig0 = sum(SIG3A[i] * sigmoid(np.float64(SIG3B[i] * SIG3C[i]))
               for i in range(NSIG))

    cem = np.zeros(B)
    cm = np.zeros(B)
    bond_num = np.zeros(B)
    for c in range(N_CORES):
        b = c // 4
        r0 = (c % 4) * ROWS_PER_CORE
        o = np.asarray(outs[c]["out"], np.float64)
        m = atom_mask[b, r0:r0 + ROWS_PER_CORE].reshape(SUBS, 128).T  # [128,S]
        for s in range(SUBS):
            ssum = sum(SIG3A[i] * o[:, s * NSIG + i] for i in range(NSIG))
            cem[b] += (m[:, s] * (ssum - sig0)).sum()
            cnt = NA - o[:, 12 + s]                     # cols with c=1 (incl diag)
            cm[b] += (m[:, s] * (cnt - 1.0)).sum()
        bond_num[b] += np.asarray(outs[c]["outb"], np.float64).sum()

    lddt = cem / cm
    l_lddt = 1.0 - lddt

    # bond denominator: sum bmask = cnt' btok cnt
    cnt_tok = Am.sum(1)  # [B, NT]
    bond_den = np.einsum('bi,bij,bj->b', cnt_tok, btok_m, cnt_tok)
    l_bond = bond_num / bond_den

    # mse (host, f64)
    w_tok = (1.0 + np.asarray(inputs["is_dna"], np.float64) * ALPHA_DNA
             + np.asarray(inputs["is_rna"], np.float64) * ALPHA_RNA
             + np.asarray(inputs["is_ligand"], np.float64) * ALPHA_LIGAND)
    w = np.einsum('bat,bt->ba', A, w_tok)
    num = 0.0
    den = np.zeros(B)
    for b in range(B):
        xga = _weighted_rigid_align_np(x_gt[b], x[b], w[b], atom_mask[b])
        num += (((x[b] - xga) ** 2).sum(-1) * w[b] * atom_mask[b]).sum()
        den[b] = atom_mask[b].sum()
    l_mse = (1.0 / 3.0) * num / den

    l = WT * (l_mse + ALPHA_BOND * l_bond) + l_lddt
    return np.float32(l.mean())


import concourse.bass as bass
import concourse.bacc as bacc
import concourse.tile as tile
from concourse import mybir
from concourse.bass import _add_dep_helper

F32 = mybir.dt.float32
BF16 = mybir.dt.bfloat16
U16 = mybir.dt.uint16
AF = mybir.ActivationFunctionType
OP = mybir.AluOpType

NA = 2048
ROWS = 512
SUBS = 4
NJ = 4          # 512-wide column chunks
BIG = 32768.0
SIG3B = (1.00862224, 0.9749538, 1.00085966)
SIG3C = (2.00029746, 0.75504458, 4.0009847)
NSIG = 3
OUT_COLS = 16
KD = 15         # split-matmul contraction rows


def build_kernel():
    nc = bacc.Bacc(None, target_bir_lowering=False)

    d_lhsx = nc.dram_tensor("lhsx", [KD, ROWS], BF16, kind="ExternalInput")
    d_lhsg = nc.dram_tensor("lhsg", [KD, ROWS], BF16, kind="ExternalInput")
    d_rhsx = nc.dram_tensor("rhsx", [KD, NA], BF16, kind="ExternalInput")
    d_rhsg = nc.dram_tensor("rhsg", [KD, NA], BF16, kind="ExternalInput")
    d_nax = nc.dram_tensor("nax", [128, SUBS], F32, kind="ExternalInput")
    d_nag = nc.dram_tensor("nag", [128, SUBS], F32, kind="ExternalInput")
    d_amt = nc.dram_tensor("amt", [256, NA], BF16, kind="ExternalInput")
    d_amto = nc.dram_tensor("amt_own", [256, ROWS], BF16, kind="ExternalInput")
    d_btok = nc.dram_tensor("btok", [256, 256], BF16, kind="ExternalInput")
    d_thr = nc.dram_tensor("thrpk", [128, SUBS], F32, kind="ExternalInput")
    d_out = nc.dram_tensor("out", [128, OUT_COLS], F32, kind="ExternalOutput")
    d_outb = nc.dram_tensor("outb", [1, 512], F32, kind="ExternalOutput")

    with tile.TileContext(nc) as tc, ExitStack() as ctx:
        const = ctx.enter_context(tc.tile_pool(name="const", bufs=1))
        big = ctx.enter_context(tc.tile_pool(name="big", bufs=1))
        work = ctx.enter_context(tc.tile_pool(name="work", bufs=3))
        scrap = ctx.enter_context(tc.tile_pool(name="scrap", bufs=3))
        pab = ctx.enter_context(
            tc.tile_pool(name="pab", bufs=2, space=bass.MemorySpace.PSUM))
        psx = ctx.enter_context(
            tc.tile_pool(name="psx", bufs=4, space=bass.MemorySpace.PSUM))

        # ---- input loads ----
        LX = const.tile([KD, ROWS], BF16)
        LG = const.tile([KD, ROWS], BF16)
        RX = const.tile([KD, NA], BF16)
        RG = const.tile([KD, NA], BF16)
        NAX = const.tile([128, SUBS], F32)
        NAG = const.tile([128, SUBS], F32)
        AMT = [const.tile([128, NA], BF16, name=f"amt{i}") for i in range(2)]
        AMTO = [const.tile([128, ROWS], BF16, name=f"amto{i}") for i in range(2)]
        BT = [const.tile([128, 256], BF16, name=f"bt{i}") for i in range(2)]
        THR = const.tile([128, SUBS], F32)
        OUTACC = const.tile([128, OUT_COLS], F32)
        OUTB = const.tile([1, 512], F32)
        TBIAS = const.tile([128, NSIG], F32)
        ONES4 = const.tile([128, SUBS], BF16)
        for k in range(NSIG):
            nc.vector.memset(TBIAS[:, k:k + 1], float(SIG3B[k] * SIG3C[k]))
        nc.vector.memset(ONES4[:], 1.0)
        nc.vector.memset(OUTACC[:], 0.0)

        nc.sync.dma_start(LX[:], d_lhsx[:])
        nc.sync.dma_start(LG[:], d_lhsg[:])
        nc.sync.dma_start(RX[:], d_rhsx[:])
        nc.sync.dma_start(RG[:], d_rhsg[:])
        nc.sync.dma_start(NAX[:], d_nax[:])
        nc.sync.dma_start(NAG[:], d_nag[:])
        for i in range(2):
            nc.sync.dma_start(AMTO[i][:], d_amto[i * 128:(i + 1) * 128, :])
            nc.sync.dma_start(BT[i][:], d_btok[i * 128:(i + 1) * 128, :])
        nc.sync.dma_start(THR[:], d_thr[:])
        # amt (largest input) is only needed by phase-B bond matmuls
        for i in range(2):
            nc.sync.dma_start(AMT[i][:], d_amt[i * 128:(i + 1) * 128, :])

        # persistent per-sub SBUF tensors
        DX = [big.tile([128, NA], F32, name=f"dx{s}") for s in range(SUBS)]
        DG = [big.tile([128, NA], F32, name=f"dg{s}") for s in range(SUBS)]
        DFA = [big.tile([128, NA], BF16, name=f"dfa{s}") for s in range(SUBS)]
        RB = [[big.tile([128, 128], BF16, name=f"rb{s}_{t}") for t in range(2)]
              for s in range(SUBS)]

        # ---- phase A0: bond token matmuls (weights BT[k] reused across s) --
        for th in range(2):
            thc = slice(th * 128, (th + 1) * 128)
            P1s = [psx.tile([128, 512], F32, tag="pcx", name=f"p1_{th}_{s}")
                   for s in range(SUBS)]
            for k in range(2):
                for s in range(SUBS):
                    sc = slice(s * 128, (s + 1) * 128)
                    nc.tensor.matmul(P1s[s][:, 0:128], BT[k][:, thc],
                                     AMTO[k][:, sc],
                                     start=(k == 0), stop=(k == 1))
            for s in range(SUBS):
                nc.vector.tensor_copy(RB[s][th][:], P1s[s][:, 0:128])

        # ---- phase A1: dist matmuls (bf16 split, K=15) + sqrt + diff -------
        sqrt_insts = []
        for s in range(SUBS):
            sc = slice(s * 128, (s + 1) * 128)
            for half, (L, R, D, NB) in enumerate(
                    [(LX, RX, DX, NAX), (LG, RG, DG, NAG)]):
                for hp in range(2):
                    PH = pab.tile([128, 1024], F32, tag="pab")
                    for jj in range(2):
                        j = hp * 2 + jj
                        jc = slice(j * 512, (j + 1) * 512)
                        nc.tensor.matmul(PH[:, jj * 512:(jj + 1) * 512],
                                         L[:, sc], R[:, jc],
                                         start=True, stop=True)
                    sqrt_insts.append(nc.scalar.activation(
                        D[s][:, hp * 1024:(hp + 1) * 1024], PH[:], AF.Sqrt,
                        bias=NB[:, s:s + 1]))
        last_sqrt = sqrt_insts[-1]

        # ---- phase B1: sigma-feeding chains for all subs (DVE -> ACT) -----
        for s in range(SUBS):
            DF = work.tile([128, NA], BF16, tag="df")
            nc.vector.tensor_tensor(DF[:], DX[s][:], DG[s][:], OP.subtract)
            nc.vector.tensor_scalar(
                DFA[s][:].bitcast(U16), DF[:].bitcast(U16), 0x7FFF, None,
                OP.bitwise_and)
            QC = work.tile([128, NA], BF16, tag="qc")
            nc.vector.tensor_scalar(
                QC[:], DG[s][:], THR[:, s:s + 1], None, OP.is_ge, OP.add,
                accum_out=OUTACC[:, 12 + s:13 + s])
            Q = work.tile([128, NA], BF16, tag="q")
            nc.vector.tensor_scalar(Q[:], QC[:], BIG, None, OP.mult)
            DP = work.tile([128, NA], BF16, tag=f"dp{s}")
            nc.vector.tensor_tensor(DP[:], DFA[s][:], Q[:], OP.add)
            for k in range(NSIG):
                SG = scrap.tile([128, NA], BF16, tag="sg")
                si = nc.scalar.activation(
                    SG[:], DP[:], AF.Sigmoid, bias=TBIAS[:, k:k + 1],
                    scale=-float(SIG3B[k]),
                    accum_out=OUTACC[:, s * NSIG + k:s * NSIG + k + 1])
                _add_dep_helper(si.ins, last_sqrt.ins, sync=False,
                                reason="group ACT table usage")

        # ---- phase B2: bond matrix, products, PE reduction ----------------
        first_rmm = True
        PRB = None
        for s in range(SUBS):
            PCs = [psx.tile([128, 512], F32, tag="pcx", name=f"pc_{s}_{j}")
                   for j in range(NJ)]
            for th in range(2):
                for j in range(NJ):
                    jc = slice(j * 512, (j + 1) * 512)
                    nc.tensor.matmul(PCs[j][:], RB[s][th][:], AMT[th][:, jc],
                                     start=(th == 0), stop=(th == 1),
                                     skip_group_check=True)
            if PRB is None:
                PRB = pab.tile([1, 512], F32, tag="pab")
            for j in range(NJ):
                jc = slice(j * 512, (j + 1) * 512)
                H = scrap.tile([128, 512], BF16, tag="h")
                nc.vector.tensor_tensor(
                    H[:], PCs[j][:], DFA[s][:, jc], OP.mult)
                H2 = scrap.tile([128, 512], BF16, tag="h2")
                nc.vector.tensor_tensor(H2[:], H[:], H[:], OP.mult)
                nc.tensor.matmul(
                    PRB[:], ONES4[:, 0:1], H2[:],
                    start=first_rmm, stop=(s == SUBS - 1 and j == NJ - 1),
                    skip_group_check=True)
                first_rmm = False
            if s == SUBS - 1:
                nc.scalar.activation(OUTB[:], PRB[:], AF.Copy)

        nc.sync.dma_start(d_out[:], OUTACC[:])
        nc.sync.dma_start(d_outb[:], OUTB[:])

    nc.compile()
    return nc


_NC_CACHE = {}


def _get_nc():
    if "nc" not in _NC_CACHE:
        _NC_CACHE["nc"] = build_kernel()
    return _NC_CACHE["nc"]


def kernel(x, x_gt, atom_mask, atom_to_token_index, token_bonds,
           is_polymer, is_ligand, is_dna, is_rna):
    from concourse import bass_utils

    in_maps, ctx = pack_inputs(x, x_gt, atom_mask, atom_to_token_index,
                               token_bonds, is_polymer, is_ligand,
                               is_dna, is_rna)
    nc = _get_nc()
    res = bass_utils.run_bass_kernel_spmd(
        nc, in_maps, core_ids=list(range(N_CORES)))
    outs = [res.results[c] for c in range(N_CORES)]
    inputs = dict(x=x, x_gt=x_gt, atom_mask=atom_mask,
                  atom_to_token_index=atom_to_token_index,
                  token_bonds=token_bonds, is_polymer=is_polymer,
                  is_ligand=is_ligand, is_dna=is_dna, is_rna=is_rna)
    return assemble(outs, inputs, ctx)

